# revision 23
# baseline (speedup 1.0000x reference)
"""AttentionBlock (groupnorm -> qkv -> softmax attention -> proj -> residual)
on 8 TRN2 NeuronCores, data-parallel over batch (B=32 -> 4 per core).

Self-contained: hardcodes shapes; builds one Bass/Tile graph and runs it
SPMD on cores 0..7 via run_bass_kernel_spmd. Host-side prep (part of the
sharding step): weights pre-cast to fp8e4m3 and repacked so every DMA
descriptor is a multi-KB contiguous run per partition; x passed both
token-major (residual) and channel-major (GN path) in bf16. All five GEMMs
run in fp8 with DoubleRow perf mode. Softmax exp is computed with a
constant shift (exp(s*scale - SHIFT)) so pt stays inside fp8e4m3 range;
the shift cancels exactly in the softmax normalization.
"""

import numpy as np
import ml_dtypes
from contextlib import ExitStack

import concourse.bass as bass
import concourse.tile as tile
from concourse import bacc, mybir
from concourse.bass_utils import run_bass_kernel_spmd

F32 = mybir.dt.float32
BF16 = mybir.dt.bfloat16
F8 = mybir.dt.float8e4
I32 = mybir.dt.int32
DR = mybir.MatmulPerfMode.DoubleRow

B, H, W, C = 32, 32, 32, 512
N = H * W            # 1024 tokens
G = 8                # groups
NCORES = 8
BPC = B // NCORES    # batches per core
EPS = 1e-3
SCALE = 1.0 / float(np.sqrt(C))
SHIFT = 3.0          # softmax exp shift (cancels in normalization)
P = 128
CT = C // P          # 4 channel tiles
TT = N // P          # 8 token tiles
MQK = 2 * C // P     # 8 d-tiles for q+k


def _build(ctx: ExitStack, tc: "tile.TileContext", io: dict):
    nc = tc.nc
    xf_ext = io["x"]           # [BPC, P, TT*C] bf16 (residual, token-major packed)
    xT_ext = io["xT16"]        # [BPC, P, CT*N] bf16 (channel-major packed)
    wqkv_ext = io["wqkv8"]     # [P, CT*3C] fp8 packed
    wp_ext = io["wp8"]         # [P, CT*C] fp8 packed
    c32_ext = io["consts32"]   # [P, 16] f32: gamma(4), beta(4), bqk(8)
    c8_ext = io["consts8"]     # [P, CT] fp8: b_v columns
    bp16_ext = io["bp16"]      # [C] bf16
    out_ext = io["out"]        # [BPC, P, TT, C] f32 packed

    # ---------------- pools ----------------
    const_pool = ctx.enter_context(tc.tile_pool(name="consts", bufs=1))
    xf_pool = ctx.enter_context(tc.tile_pool(name="xf", bufs=2))
    xT_pool = ctx.enter_context(tc.tile_pool(name="xT", bufs=2))
    hT_pool = ctx.enter_context(tc.tile_pool(name="hT", bufs=2))
    qk_pool = ctx.enter_context(tc.tile_pool(name="qk", bufs=2))
    v_pool = ctx.enter_context(tc.tile_pool(name="vv", bufs=2))
    pt_pool = ctx.enter_context(tc.tile_pool(name="pt", bufs=2))
    hTn_pool = ctx.enter_context(tc.tile_pool(name="hTn", bufs=2))
    out_pool = ctx.enter_context(tc.tile_pool(name="outb", bufs=2))
    small = ctx.enter_context(tc.tile_pool(name="small", bufs=4))
    tiny = ctx.enter_context(tc.tile_pool(name="tiny", bufs=1))
    rinv_pool = ctx.enter_context(tc.tile_pool(name="rinv", bufs=2))

    psA = ctx.enter_context(tc.tile_pool(name="psA", bufs=6, space="PSUM"))
    psB = ctx.enter_context(tc.tile_pool(name="psB", bufs=2, space="PSUM"))

    # ---------------- tiny consts via memset (no DMA) ----------------
    ones_1x128 = const_pool.tile([1, P], BF16)
    nc.vector.memset(ones_1x128, 1.0)
    ones8_dr = const_pool.tile([P, 2, 16], F8)
    nc.vector.memset(ones8_dr, 1.0)
    # group mask [128, 2]: partition p -> group p//64, value 1/64 (mean-of-64)
    gmask = const_pool.tile([P, 2], F32)
    nc.vector.memset(gmask, 0.0)
    nc.vector.memset(gmask[0:64, 0:1], 1.0 / 64.0)
    nc.vector.memset(gmask[64:128, 1:2], 1.0 / 64.0)
    # broadcast-back mask [2, 128]: maskT[r, p] = (p//64 == r)
    bmaskT_np = np.zeros((2, P), dtype=np.float32)
    bmaskT_np[0, 0:64] = 1.0
    bmaskT_np[1, 64:128] = 1.0
    bmaskT = const_pool.tile([2, P], F32)
    nc.scalar.dma_start(out=bmaskT, in_=nc.inline_tensor(bmaskT_np, "bmaskT_c").ap())
    eps_sb = const_pool.tile([2, 1], F32)
    nc.vector.memset(eps_sb, EPS)
    shift_sb = const_pool.tile([P, 1], F32)
    nc.vector.memset(shift_sb, -SHIFT)
    magic = const_pool.tile([2, CT], I32)   # fast-rsqrt seed constant
    nc.vector.memset(magic, 0x5F3759DF)

    def xT_load(b):
        """channel-major x[b] on the sync HWDGE ring as 2 half ops (4KB
        contiguous per partition each — each op fans out over all 16 SDMA
        engines at full bandwidth)."""
        xT = xT_pool.tile([P, CT, N], BF16, name=f"xT{b}", tag="xT")
        src_ct = xT_ext[b].rearrange("p (ct n) -> p ct n", ct=CT)
        nc.sync.dma_start(out=xT[:, 0:2, :], in_=src_ct[:, 0:2, :])
        nc.sync.dma_start(out=xT[:, 2:4, :], in_=src_ct[:, 2:4, :])
        return xT

    def xf_load(b):
        """token-major x[b] (residual source), 8KB per partition, one op."""
        xf = xf_pool.tile([P, TT, C], BF16, name=f"xf{b}", tag="xf")
        nc.sync.dma_start(out=xf,
                          in_=xf_ext[b].rearrange("p (t c) -> p t c", t=TT))
        return xf

    # batch 0's xT issues first and owns the full DMA bandwidth; weights and
    # xf queue up behind it on the same FIFO ring
    xts = {0: xT_load(0)}

    # warm the ACT table (exp set — the only set the kernel ever needs, since
    # GN's rsqrt runs on DVE) inside the DMA shadow
    warm = small.tile([2, 1], F32, tag="warm")
    nc.scalar.activation(warm, eps_sb, mybir.ActivationFunctionType.Exp,
                         scale=-0.5)

    # ---------------- weights / consts (fp8, packed layouts) ----------------
    wqkv = const_pool.tile([P, CT, 3 * C], F8)
    nc.sync.dma_start(out=wqkv,
                      in_=wqkv_ext.rearrange("p (kt d) -> p kt d", kt=CT))
    wp = const_pool.tile([P, CT, C], F8)
    nc.sync.dma_start(out=wp, in_=wp_ext.rearrange("p (kt d) -> p kt d", kt=CT))

    c32 = const_pool.tile([P, 16], F32)
    nc.scalar.dma_start(out=c32, in_=c32_ext)
    gamma_sb = c32[:, 0:CT]
    beta_sb = c32[:, CT:2 * CT]
    bqk_cols = c32[:, 8:8 + MQK]

    bv8_cols = const_pool.tile([P, CT], F8)
    nc.scalar.dma_start(out=bv8_cols, in_=c8_ext)

    bp16 = const_pool.tile([1, C], BF16)
    nc.scalar.dma_start(
        out=bp16,
        in_=bass.AP(tensor=bp16_ext.tensor, offset=bp16_ext.offset,
                    ap=[[0, 1], [1, C]]),
    )

    # residual source for batch 0 loads behind the weights on the sync ring
    xfs = {0: xf_load(0)}

    bp_bcast = const_pool.tile([P, C], BF16)

    def emit_bp_fold():
        # bp_bcast = broadcast(bv @ W_p + b_proj); emitted mid-batch-0 so the
        # wp-DMA dependency never stalls the front of the PE queue
        ps_bv = psB.tile([1, 512], F32, tag="ps_small")
        for ct in range(CT):
            nc.tensor.matmul(ps_bv, lhsT=bv8_cols[:, ct:ct + 1],
                             rhs=wp[:, ct, :],
                             start=(ct == 0), stop=(ct == CT - 1))
        bp_eff = const_pool.tile([1, C], BF16)
        nc.vector.tensor_add(bp_eff, ps_bv, bp16)
        ps_bc2 = psA.tile([P, 512], F32, tag="ps")
        nc.tensor.matmul(ps_bc2, lhsT=ones_1x128, rhs=bp_eff,
                         start=True, stop=True)
        nc.scalar.copy(bp_bcast, ps_bc2)

    for b in range(BPC):
        # ---------------- load ----------------
        xT = xts.pop(b)
        xf = xfs.pop(b)
        if b > 0:
            # fold (b_v @ W_p + b_proj) into the residual source
            nc.vector.tensor_add(
                xf, xf,
                bass.AP(tensor=bp_bcast.tensor, offset=bp_bcast.offset,
                        ap=[bp_bcast.ap[0], [0, TT], [1, C]]),
            )

        # ---------------- groupnorm stats (batched across c-tiles) --------
        mv = small.tile([P, CT, 2], F32, tag="mv")  # per-channel [mean, var]
        for ct in range(CT):
            st = small.tile([P, 2, 6], F32, tag="st")
            nc.vector.bn_stats(st[:, 0, :], xT[:, ct, 0:512])
            nc.vector.bn_stats(st[:, 1, :], xT[:, ct, 512:1024])
            nc.vector.bn_aggr(mv[:, ct, :], st)
        q2 = small.tile([P, CT, 2], F32, tag="q2")  # [mean, E[x^2]]
        nc.vector.tensor_mul(q2[:, :, 1], mv[:, :, 0], mv[:, :, 0])
        nc.vector.tensor_add(q2[:, :, 1], q2[:, :, 1], mv[:, :, 1])
        nc.vector.tensor_copy(q2[:, :, 0], mv[:, :, 0])
        ps_st = psB.tile([2, 8], F32, tag="ps_small")  # [g, (ct, stat)]
        nc.tensor.matmul(ps_st, lhsT=gmask, rhs=q2, start=True, stop=True)

        st_sb = small.tile([2, CT, 2], F32, tag="st_sb")
        nc.vector.tensor_copy(st_sb, ps_st)
        gmean = st_sb[:, :, 0]    # [2, 4] group means
        gm2 = st_sb[:, :, 1]      # [2, 4] group E[x^2]
        rsm = small.tile([2, CT, 2], F32, tag="rsm")  # [:,ct,0]=rstd [:,ct,1]=mean
        var24 = rsm[:, :, 0]
        nc.vector.tensor_mul(var24, gmean, gmean)
        nc.vector.tensor_sub(var24, gm2, var24)
        nc.vector.tensor_scalar(out=var24, in0=var24, scalar1=EPS,
                                scalar2=None, op0=mybir.AluOpType.add)
        # rstd = rsqrt(var+eps) on DVE (bit-trick seed + 2 Newton steps, 8
        # values) — keeps the ACT engine on the exp set only, avoiding the
        # per-batch table reloads that Ln/Exp would trigger
        iv = small.tile([2, CT], I32, tag="iv")
        nc.vector.tensor_scalar(out=iv, in0=var24.bitcast(I32), scalar1=1,
                                scalar2=None,
                                op0=mybir.AluOpType.logical_shift_right)
        nc.vector.tensor_sub(iv, magic, iv)
        y = iv.bitcast(F32)
        t1 = small.tile([2, CT], F32, tag="t1")
        for _ in range(2):
            nc.vector.tensor_mul(t1, var24, y)
            nc.vector.tensor_mul(t1, t1, y)
            nc.vector.tensor_scalar(out=t1, in0=t1, scalar1=-0.5, scalar2=1.5,
                                    op0=mybir.AluOpType.mult,
                                    op1=mybir.AluOpType.add)
            nc.vector.tensor_mul(y, y, t1)
        nc.vector.tensor_copy(var24, y)
        nc.vector.tensor_copy(rsm[:, :, 1], gmean)

        ps_pc = psB.tile([P, CT, 2], F32, tag="ps_small")  # [rstd_c, mean_c]
        nc.tensor.matmul(ps_pc, lhsT=bmaskT, rhs=rsm, start=True, stop=True)
        A_sb = small.tile([P, CT], F32, tag="A")
        B_sb = small.tile([P, CT], F32, tag="B")
        nc.vector.tensor_mul(A_sb, ps_pc[:, :, 0], gamma_sb)
        nc.vector.tensor_mul(B_sb, ps_pc[:, :, 1], A_sb)
        nc.vector.tensor_sub(B_sb, beta_sb, B_sb)

        # ---------------- normalize: hT = xT*A + B (fp8) ----------------
        # steady state: ct0/1 on vector (feeds the first qkT matmuls asap),
        # ct2/3 on gpsimd (concurrent, frees vector time); batch 0 is
        # latency-critical so everything stays on the faster vector engine
        hT = hT_pool.tile([P, CT, N], F8, name=f"hT{b}", tag="hT")
        for ct in range(CT):
            eng = nc.vector if (ct < 2 or b == 0) else nc.gpsimd
            eng.tensor_scalar(
                out=hT[:, ct, :], in0=xT[:, ct, :],
                scalar1=A_sb[:, ct:ct + 1], scalar2=B_sb[:, ct:ct + 1],
                op0=mybir.AluOpType.mult, op1=mybir.AluOpType.add,
            )

        # ---------------- qkT = (W_qk)^T @ hT  [d-major, fp8 DR] ----------
        qk = qk_pool.tile([P, MQK, N], F8, name=f"qk{b}", tag="qk")
        for m in range(MQK):
            ps0 = psA.tile([P, 512], F32, tag="ps")
            ps1 = psA.tile([P, 512], F32, tag="ps")
            for kc in (0, 2):
                lw = wqkv[:, kc:kc + 2, m * P:(m + 1) * P]
                nc.tensor.matmul(ps0, lhsT=lw, rhs=hT[:, kc:kc + 2, 0:512],
                                 start=(kc == 0), stop=(kc == 2), perf_mode=DR)
                nc.tensor.matmul(ps1, lhsT=lw, rhs=hT[:, kc:kc + 2, 512:1024],
                                 start=(kc == 0), stop=(kc == 2), perf_mode=DR)
            nc.scalar.activation(qk[:, m, 0:512], ps0,
                                 mybir.ActivationFunctionType.Identity,
                                 bias=bqk_cols[:, m:m + 1])
            nc.scalar.activation(qk[:, m, 512:1024], ps1,
                                 mybir.ActivationFunctionType.Identity,
                                 bias=bqk_cols[:, m:m + 1])

        # prefetch next batch's x behind this batch's compute
        if b + 1 < BPC:
            xts[b + 1] = xT_load(b + 1)
            xfs[b + 1] = xf_load(b + 1)

        # ---------------- v = hT^T @ W_v  [token-major, fp8 DR] -----------
        vv = v_pool.tile([P, TT, C], F8, name=f"vv{b}", tag="vv")
        for m in range(TT):
            ps = psA.tile([P, 512], F32, tag="ps")
            for kc in (0, 2):
                nc.tensor.matmul(ps, lhsT=hT[:, kc:kc + 2, m * P:(m + 1) * P],
                                 rhs=wqkv[:, kc:kc + 2, 1024:1536],
                                 start=(kc == 0), stop=(kc == 2), perf_mode=DR)
            nc.vector.tensor_copy(vv[:, m, :], ps)

        if b == 0:
            emit_bp_fold()
            nc.vector.tensor_add(
                xf, xf,
                bass.AP(tensor=bp_bcast.tensor, offset=bp_bcast.offset,
                        ap=[bp_bcast.ap[0], [0, TT], [1, C]]),
            )

        # ---------------- scoresT + exp: pt[keys, queries] (fp8 DR) -------
        pt = pt_pool.tile([P, TT, N], F8, name=f"pt{b}", tag="pt")
        for mk in range(TT):
            ps0 = psA.tile([P, 512], F32, tag="ps")
            ps1 = psA.tile([P, 512], F32, tag="ps")
            for cc in (0, 2):
                lw = qk[:, 4 + cc:4 + cc + 2, mk * P:(mk + 1) * P]  # kT block
                nc.tensor.matmul(ps0, lhsT=lw, rhs=qk[:, cc:cc + 2, 0:512],
                                 start=(cc == 0), stop=(cc == 2), perf_mode=DR)
                nc.tensor.matmul(ps1, lhsT=lw, rhs=qk[:, cc:cc + 2, 512:1024],
                                 start=(cc == 0), stop=(cc == 2), perf_mode=DR)
            nc.scalar.activation(pt[:, mk, 0:512], ps0,
                                 mybir.ActivationFunctionType.Exp,
                                 bias=shift_sb, scale=SCALE)
            nc.scalar.activation(pt[:, mk, 512:1024], ps1,
                                 mybir.ActivationFunctionType.Exp,
                                 bias=shift_sb, scale=SCALE)

        # ---------------- softmax denominator r[q] = sum_keys pt (fp8 DR) -
        ps_r0 = psA.tile([1, 512], F32, tag="ps")
        ps_r1 = psA.tile([1, 512], F32, tag="ps")
        for mk in (0, 2, 4, 6):
            nc.tensor.matmul(ps_r0, lhsT=ones8_dr[:, :, 0:1],
                             rhs=pt[:, mk:mk + 2, 0:512],
                             start=(mk == 0), stop=(mk == 6), perf_mode=DR)
            nc.tensor.matmul(ps_r1, lhsT=ones8_dr[:, :, 0:1],
                             rhs=pt[:, mk:mk + 2, 512:1024],
                             start=(mk == 0), stop=(mk == 6), perf_mode=DR)
        r16 = tiny.tile([1, N], BF16, tag="r16")
        nc.scalar.copy(r16[:, 0:512], ps_r0)
        nc.scalar.copy(r16[:, 512:1024], ps_r1)
        rb = rinv_pool.tile([P, N], F32, name=f"rb{b}", tag="rb")
        for chunk in range(2):
            ps_b = psA.tile([P, 512], F32, tag="ps")
            nc.tensor.matmul(ps_b, lhsT=ones_1x128,
                             rhs=r16[0:1, chunk * 512:(chunk + 1) * 512],
                             start=True, stop=True)
            # 128-partition-parallel fast reciprocal (psum -> sbuf f32)
            nc.vector.reciprocal_approx_fast(
                out=rb[:, chunk * 512:(chunk + 1) * 512], in_=ps_b)

        # ---------------- hTn = (v^T @ pt) * rb  [channel-major, fp8] -----
        hTn = hTn_pool.tile([P, CT, N], F8, name=f"hTn{b}", tag="hTn")
        for mc in range(CT):
            ps0 = psA.tile([P, 512], F32, tag="ps")
            ps1 = psA.tile([P, 512], F32, tag="ps")
            for mk in (0, 2, 4, 6):
                lw = vv[:, mk:mk + 2, mc * P:(mc + 1) * P]
                nc.tensor.matmul(ps0, lhsT=lw, rhs=pt[:, mk:mk + 2, 0:512],
                                 start=(mk == 0), stop=(mk == 6), perf_mode=DR)
                nc.tensor.matmul(ps1, lhsT=lw, rhs=pt[:, mk:mk + 2, 512:1024],
                                 start=(mk == 0), stop=(mk == 6), perf_mode=DR)
            nc.vector.tensor_mul(hTn[:, mc, 0:512], ps0, rb[:, 0:512])
            nc.vector.tensor_mul(hTn[:, mc, 512:1024], ps1, rb[:, 512:1024])

        # ---------------- proj + residual -> out [token-major, fp8 DR] ----
        # outb in bf16: halves DVE add cost and output DMA bytes; the host
        # upcasts to f32 (the reference output magnitude is O(1), so bf16
        # rounding adds ~0.2% relative error)
        outb = out_pool.tile([P, TT, C], BF16, name=f"outb{b}", tag="outb")
        for m in range(TT):
            ps = psA.tile([P, 512], F32, tag="ps")
            for mc in (0, 2):
                nc.tensor.matmul(ps, lhsT=hTn[:, mc:mc + 2, m * P:(m + 1) * P],
                                 rhs=wp[:, mc:mc + 2, :], start=(mc == 0),
                                 stop=(mc == 2), perf_mode=DR)
            nc.vector.tensor_add(outb[:, m, :], ps, xf[:, m, :])
            if m % 2 == 1:
                nc.gpsimd.dma_start(out=out_ext[b][:, m - 1:m + 1, :],
                                    in_=outb[:, m - 1:m + 1, :])


_CACHED_NC = None


def _build_nc():
    global _CACHED_NC
    if _CACHED_NC is not None:
        return _CACHED_NC
    nc = bacc.Bacc("TRN2", target_bir_lowering=False, debug=False,
                   num_devices=NCORES)
    io = {
        "x": nc.dram_tensor("x", [BPC, P, TT * C], BF16,
                            kind="ExternalInput").ap(),
        "xT16": nc.dram_tensor("xT16", [BPC, P, CT * N], BF16,
                               kind="ExternalInput").ap(),
        "wqkv8": nc.dram_tensor("wqkv8", [P, CT * 3 * C], F8,
                                kind="ExternalInput").ap(),
        "wp8": nc.dram_tensor("wp8", [P, CT * C], F8,
                              kind="ExternalInput").ap(),
        "consts32": nc.dram_tensor("consts32", [P, 16], F32,
                                   kind="ExternalInput").ap(),
        "consts8": nc.dram_tensor("consts8", [P, CT], F8,
                                  kind="ExternalInput").ap(),
        "bp16": nc.dram_tensor("bp16", [C], BF16, kind="ExternalInput").ap(),
        "out": nc.dram_tensor("out", [BPC, P, TT, C], BF16,
                              kind="ExternalOutput").ap(),
    }
    with tile.TileContext(nc) as tc:
        with ExitStack() as ctx:
            _build(ctx, tc, io)
    nc.compile()
    _CACHED_NC = nc
    return nc


def _run(inputs: dict, trace: bool = False):
    nc = _build_nc()
    x = np.ascontiguousarray(inputs["x"], dtype=np.float32).reshape(B, N, C)
    x16 = x.astype(ml_dtypes.bfloat16)
    # token-major packed: [B, P, TT*C], partition p <- token t*128+p
    xf_p = np.ascontiguousarray(
        x16.reshape(B, TT, P, C).transpose(0, 2, 1, 3)).reshape(B, P, TT * C)
    # channel-major packed: [B, P, CT*N], partition p <- channel ct*128+p
    xT16_full = np.ascontiguousarray(
        x.transpose(0, 2, 1).reshape(B, CT, P, N).transpose(0, 2, 1, 3)
    ).astype(ml_dtypes.bfloat16).reshape(B, P, CT * N)

    w8 = np.ascontiguousarray(inputs["w_qkv"], dtype=np.float32).astype(
        ml_dtypes.float8_e4m3)
    wqkv_p = np.ascontiguousarray(
        w8.reshape(CT, P, 3 * C).transpose(1, 0, 2)).reshape(P, CT * 3 * C)
    wp8 = np.ascontiguousarray(inputs["w_proj"], dtype=np.float32).astype(
        ml_dtypes.float8_e4m3)
    wp_p = np.ascontiguousarray(
        wp8.reshape(CT, P, C).transpose(1, 0, 2)).reshape(P, CT * C)

    gamma = np.asarray(inputs["gamma"], dtype=np.float32)
    beta = np.asarray(inputs["beta"], dtype=np.float32)
    bqkv = np.asarray(inputs["b_qkv"], dtype=np.float32)
    consts32 = np.zeros((P, 16), dtype=np.float32)
    consts32[:, 0:CT] = gamma.reshape(CT, P).T
    consts32[:, CT:2 * CT] = beta.reshape(CT, P).T
    consts32[:, 8:8 + MQK] = bqkv[:2 * C].reshape(MQK, P).T
    consts8 = np.ascontiguousarray(
        bqkv[2 * C:].astype(ml_dtypes.float8_e4m3).reshape(CT, P).T)

    shared = {
        "wqkv8": wqkv_p,
        "wp8": wp_p,
        "consts32": consts32,
        "consts8": consts8,
        "bp16": np.asarray(inputs["b_proj"], dtype=np.float32)
            .astype(ml_dtypes.bfloat16),
    }
    in_maps = []
    for i in range(NCORES):
        m = {"x": xf_p[i * BPC:(i + 1) * BPC],
             "xT16": xT16_full[i * BPC:(i + 1) * BPC]}
        m.update(shared)
        in_maps.append(m)
    res = run_bass_kernel_spmd(nc, in_maps, list(range(NCORES)), trace=trace)
    outs = [res.results[i]["out"] for i in range(NCORES)]   # [BPC, P, TT, C] bf16
    full = np.concatenate(outs, axis=0)                     # [B, P, TT, C]
    full = full.transpose(0, 2, 1, 3).reshape(B, H, W, C).astype(np.float32)
    return full, res


def kernel(**inputs) -> np.ndarray:
    full, _ = _run(inputs, trace=False)
    return full


# revision 27
# speedup vs baseline: 1.0229x; 1.0229x over previous
"""AttentionBlock (groupnorm -> qkv -> softmax attention -> proj -> residual)
on 8 TRN2 NeuronCores, data-parallel over batch (B=32 -> 4 per core).

Self-contained: hardcodes shapes; builds one Bass/Tile graph and runs it
SPMD on cores 0..7 via run_bass_kernel_spmd. Host-side prep (part of the
sharding step): weights pre-cast to fp8e4m3 and repacked so every DMA
descriptor is a multi-KB contiguous run per partition; x passed both
token-major (residual) and channel-major (GN path) in bf16. All five GEMMs
run in fp8 with DoubleRow perf mode. Softmax exp is computed with a
constant shift (exp(s*scale - SHIFT)) so pt stays inside fp8e4m3 range;
the shift cancels exactly in the softmax normalization.
"""

import numpy as np
import ml_dtypes
from contextlib import ExitStack

import concourse.bass as bass
import concourse.tile as tile
from concourse import bacc, mybir
from concourse.bass_utils import run_bass_kernel_spmd

F32 = mybir.dt.float32
BF16 = mybir.dt.bfloat16
F8 = mybir.dt.float8e4
I32 = mybir.dt.int32
DR = mybir.MatmulPerfMode.DoubleRow

B, H, W, C = 32, 32, 32, 512
N = H * W            # 1024 tokens
G = 8                # groups
NCORES = 8
BPC = B // NCORES    # batches per core
EPS = 1e-3
SCALE = 1.0 / float(np.sqrt(C))
SHIFT = 3.0          # softmax exp shift (cancels in normalization)
P = 128
CT = C // P          # 4 channel tiles
TT = N // P          # 8 token tiles
MQK = 2 * C // P     # 8 d-tiles for q+k


def _build(ctx: ExitStack, tc: "tile.TileContext", io: dict):
    nc = tc.nc
    xf_ext = io["x"]           # [BPC, P, TT*C] bf16 (residual, token-major packed)
    xT_ext = io["xT16"]        # [BPC, P, CT*N] bf16 (channel-major packed)
    wqkv_ext = io["wqkv8"]     # [P, CT*3C] fp8 packed
    wp_ext = io["wp8"]         # [P, CT*C] fp8 packed
    c32_ext = io["consts32"]   # [P, 16] f32: gamma(4), beta(4), bqk(8)
    c8_ext = io["consts8"]     # [P, CT] fp8: b_v columns
    bp16_ext = io["bp16"]      # [C] bf16
    out_ext = io["out"]        # [BPC, P, TT, C] f32 packed

    # ---------------- pools ----------------
    const_pool = ctx.enter_context(tc.tile_pool(name="consts", bufs=1))
    xf_pool = ctx.enter_context(tc.tile_pool(name="xf", bufs=2))
    xT_pool = ctx.enter_context(tc.tile_pool(name="xT", bufs=2))
    hT_pool = ctx.enter_context(tc.tile_pool(name="hT", bufs=2))
    qk_pool = ctx.enter_context(tc.tile_pool(name="qk", bufs=2))
    v_pool = ctx.enter_context(tc.tile_pool(name="vv", bufs=2))
    pt_pool = ctx.enter_context(tc.tile_pool(name="pt", bufs=2))
    hTn_pool = ctx.enter_context(tc.tile_pool(name="hTn", bufs=2))
    out_pool = ctx.enter_context(tc.tile_pool(name="outb", bufs=2))
    small = ctx.enter_context(tc.tile_pool(name="small", bufs=4))
    tiny = ctx.enter_context(tc.tile_pool(name="tiny", bufs=1))
    rinv_pool = ctx.enter_context(tc.tile_pool(name="rinv", bufs=2))

    psA = ctx.enter_context(tc.tile_pool(name="psA", bufs=6, space="PSUM"))
    psB = ctx.enter_context(tc.tile_pool(name="psB", bufs=2, space="PSUM"))

    # ---------------- tiny consts via memset (no DMA) ----------------
    ones_1x128 = const_pool.tile([1, P], BF16)
    nc.vector.memset(ones_1x128, 1.0)
    ones8_dr = const_pool.tile([P, 2, 16], F8)
    nc.vector.memset(ones8_dr, 1.0)
    # group mask [128, 2]: partition p -> group p//64, value 1/64 (mean-of-64)
    gmask = const_pool.tile([P, 2], F32)
    nc.vector.memset(gmask, 0.0)
    nc.vector.memset(gmask[0:64, 0:1], 1.0 / 64.0)
    nc.vector.memset(gmask[64:128, 1:2], 1.0 / 64.0)
    # broadcast-back mask [2, 128]: maskT[r, p] = (p//64 == r)
    bmaskT_np = np.zeros((2, P), dtype=np.float32)
    bmaskT_np[0, 0:64] = 1.0
    bmaskT_np[1, 64:128] = 1.0
    bmaskT = const_pool.tile([2, P], F32)
    nc.scalar.dma_start(out=bmaskT, in_=nc.inline_tensor(bmaskT_np, "bmaskT_c").ap())
    eps_sb = const_pool.tile([2, 1], F32)
    nc.vector.memset(eps_sb, EPS)
    shift_sb = const_pool.tile([P, 1], F32)
    nc.vector.memset(shift_sb, -SHIFT)
    magic = const_pool.tile([2, CT], I32)   # fast-rsqrt seed constant
    nc.vector.memset(magic, 0x5F3759DF)

    def xT_load(b):
        """channel-major x[b] on the sync HWDGE ring as 2 half ops (4KB
        contiguous per partition each — each op fans out over all 16 SDMA
        engines at full bandwidth)."""
        xT = xT_pool.tile([P, CT, N], BF16, name=f"xT{b}", tag="xT")
        src_ct = xT_ext[b].rearrange("p (ct n) -> p ct n", ct=CT)
        nc.sync.dma_start(out=xT[:, 0:2, :], in_=src_ct[:, 0:2, :])
        nc.sync.dma_start(out=xT[:, 2:4, :], in_=src_ct[:, 2:4, :])
        return xT

    def xf_load(b):
        """token-major x[b] (residual source), 8KB per partition, one op."""
        xf = xf_pool.tile([P, TT, C], BF16, name=f"xf{b}", tag="xf")
        nc.sync.dma_start(out=xf,
                          in_=xf_ext[b].rearrange("p (t c) -> p t c", t=TT))
        return xf

    # batch 0's xT issues first and owns the full DMA bandwidth; weights and
    # xf queue up behind it on the same FIFO ring
    xts = {0: xT_load(0)}

    # warm the ACT table (exp set — the only set the kernel ever needs, since
    # GN's rsqrt runs on DVE) inside the DMA shadow
    warm = small.tile([2, 1], F32, tag="warm")
    nc.scalar.activation(warm, eps_sb, mybir.ActivationFunctionType.Exp,
                         scale=-0.5)

    # ---------------- weights / consts (fp8, packed layouts) ----------------
    wqkv = const_pool.tile([P, CT, 3 * C], F8)
    nc.sync.dma_start(out=wqkv,
                      in_=wqkv_ext.rearrange("p (kt d) -> p kt d", kt=CT))
    wp = const_pool.tile([P, CT, C], F8)
    nc.sync.dma_start(out=wp, in_=wp_ext.rearrange("p (kt d) -> p kt d", kt=CT))

    c32 = const_pool.tile([P, 16], F32)
    nc.scalar.dma_start(out=c32, in_=c32_ext)
    gamma_sb = c32[:, 0:CT]
    beta_sb = c32[:, CT:2 * CT]
    bqk_cols = c32[:, 8:8 + MQK]

    bv8_cols = const_pool.tile([P, CT], F8)
    nc.scalar.dma_start(out=bv8_cols, in_=c8_ext)

    bp16 = const_pool.tile([1, C], BF16)
    nc.scalar.dma_start(
        out=bp16,
        in_=bass.AP(tensor=bp16_ext.tensor, offset=bp16_ext.offset,
                    ap=[[0, 1], [1, C]]),
    )

    # residual source for batch 0 loads behind the weights on the sync ring
    xfs = {0: xf_load(0)}

    bp_bcast = const_pool.tile([P, C], BF16)

    def emit_bp_fold():
        # bp_bcast = broadcast(bv @ W_p + b_proj); emitted mid-batch-0 so the
        # wp-DMA dependency never stalls the front of the PE queue
        ps_bv = psB.tile([1, 512], F32, tag="ps_small")
        for ct in range(CT):
            nc.tensor.matmul(ps_bv, lhsT=bv8_cols[:, ct:ct + 1],
                             rhs=wp[:, ct, :],
                             start=(ct == 0), stop=(ct == CT - 1))
        bp_eff = const_pool.tile([1, C], BF16)
        nc.vector.tensor_add(bp_eff, ps_bv, bp16)
        ps_bc2 = psA.tile([P, 512], F32, tag="ps")
        nc.tensor.matmul(ps_bc2, lhsT=ones_1x128, rhs=bp_eff,
                         start=True, stop=True)
        nc.scalar.copy(bp_bcast, ps_bc2)

    def stats_norm(b, xT):
        """GroupNorm stats + normalize for batch b; returns hT.

        In steady state this is emitted one batch AHEAD (during batch b-1's
        attention phase) so the long serial stats->rsqrt->normalize chain
        rides in Vector's idle window and never blocks the PE."""
        # ---------------- groupnorm stats (batched across c-tiles) --------
        mv = small.tile([P, CT, 2], F32, tag="mv")  # per-channel [mean, var]
        for ct in range(CT):
            st = small.tile([P, 2, 6], F32, tag="st")
            nc.vector.bn_stats(st[:, 0, :], xT[:, ct, 0:512])
            nc.vector.bn_stats(st[:, 1, :], xT[:, ct, 512:1024])
            nc.vector.bn_aggr(mv[:, ct, :], st)
        q2 = small.tile([P, CT, 2], F32, tag="q2")  # [mean, E[x^2]]
        nc.vector.tensor_mul(q2[:, :, 1], mv[:, :, 0], mv[:, :, 0])
        nc.vector.tensor_add(q2[:, :, 1], q2[:, :, 1], mv[:, :, 1])
        nc.vector.tensor_copy(q2[:, :, 0], mv[:, :, 0])
        ps_st = psB.tile([2, 8], F32, tag="ps_small")  # [g, (ct, stat)]
        nc.tensor.matmul(ps_st, lhsT=gmask, rhs=q2, start=True, stop=True)

        st_sb = small.tile([2, CT, 2], F32, tag="st_sb")
        nc.vector.tensor_copy(st_sb, ps_st)
        gmean = st_sb[:, :, 0]    # [2, 4] group means
        gm2 = st_sb[:, :, 1]      # [2, 4] group E[x^2]
        rsm = small.tile([2, CT, 2], F32, tag="rsm")  # [:,ct,0]=rstd [:,ct,1]=mean
        var24 = rsm[:, :, 0]
        nc.vector.tensor_mul(var24, gmean, gmean)
        nc.vector.tensor_sub(var24, gm2, var24)
        nc.vector.tensor_scalar(out=var24, in0=var24, scalar1=EPS,
                                scalar2=None, op0=mybir.AluOpType.add)
        # rstd = rsqrt(var+eps) on DVE (bit-trick seed + 2 Newton steps, 8
        # values) — keeps the ACT engine on the exp set only, avoiding the
        # per-batch table reloads that Ln/Exp would trigger
        iv = small.tile([2, CT], I32, tag="iv")
        nc.vector.tensor_scalar(out=iv, in0=var24.bitcast(I32), scalar1=1,
                                scalar2=None,
                                op0=mybir.AluOpType.logical_shift_right)
        nc.vector.tensor_sub(iv, magic, iv)
        y = iv.bitcast(F32)
        t1 = small.tile([2, CT], F32, tag="t1")
        for _ in range(2):
            nc.vector.tensor_mul(t1, var24, y)
            nc.vector.tensor_mul(t1, t1, y)
            nc.vector.tensor_scalar(out=t1, in0=t1, scalar1=-0.5, scalar2=1.5,
                                    op0=mybir.AluOpType.mult,
                                    op1=mybir.AluOpType.add)
            nc.vector.tensor_mul(y, y, t1)
        nc.vector.tensor_copy(var24, y)
        nc.vector.tensor_copy(rsm[:, :, 1], gmean)

        ps_pc = psB.tile([P, CT, 2], F32, tag="ps_small")  # [rstd_c, mean_c]
        nc.tensor.matmul(ps_pc, lhsT=bmaskT, rhs=rsm, start=True, stop=True)
        A_sb = small.tile([P, CT], F32, tag="A")
        B_sb = small.tile([P, CT], F32, tag="B")
        nc.vector.tensor_mul(A_sb, ps_pc[:, :, 0], gamma_sb)
        nc.vector.tensor_mul(B_sb, ps_pc[:, :, 1], A_sb)
        nc.vector.tensor_sub(B_sb, beta_sb, B_sb)

        # ---------------- normalize: hT = xT*A + B (fp8) ----------------
        # steady state: ct0/1 on vector (feeds the first qkT matmuls asap),
        # ct2/3 on gpsimd (concurrent, frees vector time); batch 0 is
        # latency-critical so everything stays on the faster vector engine
        hT = hT_pool.tile([P, CT, N], F8, name=f"hT{b}", tag="hT")
        for ct in range(CT):
            eng = nc.vector if (ct < 2 or b == 0) else nc.gpsimd
            eng.tensor_scalar(
                out=hT[:, ct, :], in0=xT[:, ct, :],
                scalar1=A_sb[:, ct:ct + 1], scalar2=B_sb[:, ct:ct + 1],
                op0=mybir.AluOpType.mult, op1=mybir.AluOpType.add,
            )
        return hT

    def fold_residual(xf):
        # fold (b_v @ W_p + b_proj) into the residual source
        nc.vector.tensor_add(
            xf, xf,
            bass.AP(tensor=bp_bcast.tensor, offset=bp_bcast.offset,
                    ap=[bp_bcast.ap[0], [0, TT], [1, C]]),
        )

    hTs = {0: stats_norm(0, xts[0])}

    for b in range(BPC):
        xT = xts.pop(b)
        xf = xfs.pop(b)
        hT = hTs.pop(b)

        # ---------------- qkT = (W_qk)^T @ hT  [d-major, fp8 DR] ----------
        qk = qk_pool.tile([P, MQK, N], F8, name=f"qk{b}", tag="qk")
        for m in range(MQK):
            ps0 = psA.tile([P, 512], F32, tag="ps")
            ps1 = psA.tile([P, 512], F32, tag="ps")
            for kc in (0, 2):
                lw = wqkv[:, kc:kc + 2, m * P:(m + 1) * P]
                nc.tensor.matmul(ps0, lhsT=lw, rhs=hT[:, kc:kc + 2, 0:512],
                                 start=(kc == 0), stop=(kc == 2), perf_mode=DR)
                nc.tensor.matmul(ps1, lhsT=lw, rhs=hT[:, kc:kc + 2, 512:1024],
                                 start=(kc == 0), stop=(kc == 2), perf_mode=DR)
            nc.scalar.activation(qk[:, m, 0:512], ps0,
                                 mybir.ActivationFunctionType.Identity,
                                 bias=bqk_cols[:, m:m + 1])
            nc.scalar.activation(qk[:, m, 512:1024], ps1,
                                 mybir.ActivationFunctionType.Identity,
                                 bias=bqk_cols[:, m:m + 1])

        # prefetch next batch's x behind this batch's compute
        if b + 1 < BPC:
            xts[b + 1] = xT_load(b + 1)
            xfs[b + 1] = xf_load(b + 1)

        # ---------------- v = hT^T @ W_v  [token-major, fp8 DR] -----------
        vv = v_pool.tile([P, TT, C], F8, name=f"vv{b}", tag="vv")
        for m in range(TT):
            ps = psA.tile([P, 512], F32, tag="ps")
            for kc in (0, 2):
                nc.tensor.matmul(ps, lhsT=hT[:, kc:kc + 2, m * P:(m + 1) * P],
                                 rhs=wqkv[:, kc:kc + 2, 1024:1536],
                                 start=(kc == 0), stop=(kc == 2), perf_mode=DR)
            nc.vector.tensor_copy(vv[:, m, :], ps)

        if b == 0:
            emit_bp_fold()
            fold_residual(xf)

        # ---------------- scoresT + exp: pt[keys, queries] (fp8 DR) -------
        pt = pt_pool.tile([P, TT, N], F8, name=f"pt{b}", tag="pt")
        for mk in range(TT):
            ps0 = psA.tile([P, 512], F32, tag="ps")
            ps1 = psA.tile([P, 512], F32, tag="ps")
            for cc in (0, 2):
                lw = qk[:, 4 + cc:4 + cc + 2, mk * P:(mk + 1) * P]  # kT block
                nc.tensor.matmul(ps0, lhsT=lw, rhs=qk[:, cc:cc + 2, 0:512],
                                 start=(cc == 0), stop=(cc == 2), perf_mode=DR)
                nc.tensor.matmul(ps1, lhsT=lw, rhs=qk[:, cc:cc + 2, 512:1024],
                                 start=(cc == 0), stop=(cc == 2), perf_mode=DR)
            nc.scalar.activation(pt[:, mk, 0:512], ps0,
                                 mybir.ActivationFunctionType.Exp,
                                 bias=shift_sb, scale=SCALE)
            nc.scalar.activation(pt[:, mk, 512:1024], ps1,
                                 mybir.ActivationFunctionType.Exp,
                                 bias=shift_sb, scale=SCALE)

        # next batch's GN pipelined here: its vector work fills the idle
        # window while PE runs this batch's score matmuls, and its tiny
        # stats matmuls fill the PE's exp-wait gap before the r-sum chain
        if b + 1 < BPC:
            hTs[b + 1] = stats_norm(b + 1, xts[b + 1])
            fold_residual(xfs[b + 1])

        # ---------------- softmax denominator r[q] = sum_keys pt (fp8 DR) -
        ps_r0 = psA.tile([1, 512], F32, tag="ps")
        ps_r1 = psA.tile([1, 512], F32, tag="ps")
        for mk in (0, 2, 4, 6):
            nc.tensor.matmul(ps_r0, lhsT=ones8_dr[:, :, 0:1],
                             rhs=pt[:, mk:mk + 2, 0:512],
                             start=(mk == 0), stop=(mk == 6), perf_mode=DR)
            nc.tensor.matmul(ps_r1, lhsT=ones8_dr[:, :, 0:1],
                             rhs=pt[:, mk:mk + 2, 512:1024],
                             start=(mk == 0), stop=(mk == 6), perf_mode=DR)
        r16 = tiny.tile([1, N], BF16, tag="r16")
        nc.scalar.copy(r16[:, 0:512], ps_r0)
        nc.scalar.copy(r16[:, 512:1024], ps_r1)
        rb = rinv_pool.tile([P, N], F32, name=f"rb{b}", tag="rb")
        for chunk in range(2):
            ps_b = psA.tile([P, 512], F32, tag="ps")
            nc.tensor.matmul(ps_b, lhsT=ones_1x128,
                             rhs=r16[0:1, chunk * 512:(chunk + 1) * 512],
                             start=True, stop=True)
            # 128-partition-parallel fast reciprocal (psum -> sbuf f32)
            nc.vector.reciprocal_approx_fast(
                out=rb[:, chunk * 512:(chunk + 1) * 512], in_=ps_b)

        # ---------------- hTn = (v^T @ pt) * rb  [channel-major, fp8] -----
        hTn = hTn_pool.tile([P, CT, N], F8, name=f"hTn{b}", tag="hTn")
        for mc in range(CT):
            ps0 = psA.tile([P, 512], F32, tag="ps")
            ps1 = psA.tile([P, 512], F32, tag="ps")
            for mk in (0, 2, 4, 6):
                lw = vv[:, mk:mk + 2, mc * P:(mc + 1) * P]
                nc.tensor.matmul(ps0, lhsT=lw, rhs=pt[:, mk:mk + 2, 0:512],
                                 start=(mk == 0), stop=(mk == 6), perf_mode=DR)
                nc.tensor.matmul(ps1, lhsT=lw, rhs=pt[:, mk:mk + 2, 512:1024],
                                 start=(mk == 0), stop=(mk == 6), perf_mode=DR)
            nc.vector.tensor_mul(hTn[:, mc, 0:512], ps0, rb[:, 0:512])
            nc.vector.tensor_mul(hTn[:, mc, 512:1024], ps1, rb[:, 512:1024])

        # ---------------- proj + residual -> out [token-major, fp8 DR] ----
        # outb in bf16: halves DVE add cost and output DMA bytes; the host
        # upcasts to f32 (the reference output magnitude is O(1), so bf16
        # rounding adds ~0.2% relative error)
        outb = out_pool.tile([P, TT, C], BF16, name=f"outb{b}", tag="outb")
        for m in range(TT):
            ps = psA.tile([P, 512], F32, tag="ps")
            for mc in (0, 2):
                nc.tensor.matmul(ps, lhsT=hTn[:, mc:mc + 2, m * P:(m + 1) * P],
                                 rhs=wp[:, mc:mc + 2, :], start=(mc == 0),
                                 stop=(mc == 2), perf_mode=DR)
            nc.vector.tensor_add(outb[:, m, :], ps, xf[:, m, :])
            if m % 2 == 1:
                nc.gpsimd.dma_start(out=out_ext[b][:, m - 1:m + 1, :],
                                    in_=outb[:, m - 1:m + 1, :])


_CACHED_NC = None


def _build_nc():
    global _CACHED_NC
    if _CACHED_NC is not None:
        return _CACHED_NC
    nc = bacc.Bacc("TRN2", target_bir_lowering=False, debug=False,
                   num_devices=NCORES)
    io = {
        "x": nc.dram_tensor("x", [BPC, P, TT * C], BF16,
                            kind="ExternalInput").ap(),
        "xT16": nc.dram_tensor("xT16", [BPC, P, CT * N], BF16,
                               kind="ExternalInput").ap(),
        "wqkv8": nc.dram_tensor("wqkv8", [P, CT * 3 * C], F8,
                                kind="ExternalInput").ap(),
        "wp8": nc.dram_tensor("wp8", [P, CT * C], F8,
                              kind="ExternalInput").ap(),
        "consts32": nc.dram_tensor("consts32", [P, 16], F32,
                                   kind="ExternalInput").ap(),
        "consts8": nc.dram_tensor("consts8", [P, CT], F8,
                                  kind="ExternalInput").ap(),
        "bp16": nc.dram_tensor("bp16", [C], BF16, kind="ExternalInput").ap(),
        "out": nc.dram_tensor("out", [BPC, P, TT, C], BF16,
                              kind="ExternalOutput").ap(),
    }
    with tile.TileContext(nc) as tc:
        with ExitStack() as ctx:
            _build(ctx, tc, io)
    nc.compile()
    _CACHED_NC = nc
    return nc


def _run(inputs: dict, trace: bool = False):
    nc = _build_nc()
    x = np.ascontiguousarray(inputs["x"], dtype=np.float32).reshape(B, N, C)
    x16 = x.astype(ml_dtypes.bfloat16)
    # token-major packed: [B, P, TT*C], partition p <- token t*128+p
    xf_p = np.ascontiguousarray(
        x16.reshape(B, TT, P, C).transpose(0, 2, 1, 3)).reshape(B, P, TT * C)
    # channel-major packed: [B, P, CT*N], partition p <- channel ct*128+p
    xT16_full = np.ascontiguousarray(
        x.transpose(0, 2, 1).reshape(B, CT, P, N).transpose(0, 2, 1, 3)
    ).astype(ml_dtypes.bfloat16).reshape(B, P, CT * N)

    w8 = np.ascontiguousarray(inputs["w_qkv"], dtype=np.float32).astype(
        ml_dtypes.float8_e4m3)
    wqkv_p = np.ascontiguousarray(
        w8.reshape(CT, P, 3 * C).transpose(1, 0, 2)).reshape(P, CT * 3 * C)
    wp8 = np.ascontiguousarray(inputs["w_proj"], dtype=np.float32).astype(
        ml_dtypes.float8_e4m3)
    wp_p = np.ascontiguousarray(
        wp8.reshape(CT, P, C).transpose(1, 0, 2)).reshape(P, CT * C)

    gamma = np.asarray(inputs["gamma"], dtype=np.float32)
    beta = np.asarray(inputs["beta"], dtype=np.float32)
    bqkv = np.asarray(inputs["b_qkv"], dtype=np.float32)
    consts32 = np.zeros((P, 16), dtype=np.float32)
    consts32[:, 0:CT] = gamma.reshape(CT, P).T
    consts32[:, CT:2 * CT] = beta.reshape(CT, P).T
    consts32[:, 8:8 + MQK] = bqkv[:2 * C].reshape(MQK, P).T
    consts8 = np.ascontiguousarray(
        bqkv[2 * C:].astype(ml_dtypes.float8_e4m3).reshape(CT, P).T)

    shared = {
        "wqkv8": wqkv_p,
        "wp8": wp_p,
        "consts32": consts32,
        "consts8": consts8,
        "bp16": np.asarray(inputs["b_proj"], dtype=np.float32)
            .astype(ml_dtypes.bfloat16),
    }
    in_maps = []
    for i in range(NCORES):
        m = {"x": xf_p[i * BPC:(i + 1) * BPC],
             "xT16": xT16_full[i * BPC:(i + 1) * BPC]}
        m.update(shared)
        in_maps.append(m)
    res = run_bass_kernel_spmd(nc, in_maps, list(range(NCORES)), trace=trace)
    outs = [res.results[i]["out"] for i in range(NCORES)]   # [BPC, P, TT, C] bf16
    full = np.concatenate(outs, axis=0)                     # [B, P, TT, C]
    full = full.transpose(0, 2, 1, 3).reshape(B, H, W, C).astype(np.float32)
    return full, res


def kernel(**inputs) -> np.ndarray:
    full, _ = _run(inputs, trace=False)
    return full


# revision 32
# speedup vs baseline: 1.0860x; 1.0617x over previous
"""AttentionBlock (groupnorm -> qkv -> softmax attention -> proj -> residual)
on 8 TRN2 NeuronCores, data-parallel over batch (B=32 -> 4 per core).

Self-contained: hardcodes shapes; builds one Bass/Tile graph and runs it
SPMD on cores 0..7 via run_bass_kernel_spmd. Host-side prep (part of the
sharding step): weights pre-cast to fp8e4m3 and repacked so every DMA
descriptor is a multi-KB contiguous run per partition; x passed both
token-major (residual) and channel-major (GN path) in bf16. All five GEMMs
run in fp8 with DoubleRow perf mode. Softmax exp is computed with a
constant shift (exp(s*scale - SHIFT)) so pt stays inside fp8e4m3 range;
the shift cancels exactly in the softmax normalization.
"""

import numpy as np
import ml_dtypes
from contextlib import ExitStack

import concourse.bass as bass
import concourse.tile as tile
from concourse import bacc, mybir
from concourse.bass_utils import run_bass_kernel_spmd

F32 = mybir.dt.float32
BF16 = mybir.dt.bfloat16
F8 = mybir.dt.float8e4
I32 = mybir.dt.int32
DR = mybir.MatmulPerfMode.DoubleRow

B, H, W, C = 32, 32, 32, 512
N = H * W            # 1024 tokens
G = 8                # groups
NCORES = 8
BPC = B // NCORES    # batches per core
EPS = 1e-3
SCALE = 1.0 / float(np.sqrt(C))
SHIFT = 3.0          # softmax exp shift (cancels in normalization)
P = 128
CT = C // P          # 4 channel tiles
TT = N // P          # 8 token tiles
MQK = 2 * C // P     # 8 d-tiles for q+k


def _build(ctx: ExitStack, tc: "tile.TileContext", io: dict):
    nc = tc.nc
    xf_ext = io["x"]           # [BPC, P, TT*C] bf16 (residual, token-major packed)
    xT_ext = io["xT16"]        # [BPC, P, CT*N] bf16 (channel-major packed)
    wqkv_ext = io["wqkv8"]     # [P, CT*3C] fp8 packed
    wp_ext = io["wp8"]         # [P, CT*C] fp8 packed
    c32_ext = io["consts32"]   # [P, 16] f32: gamma(4), beta(4), bqk(8)
    c8_ext = io["consts8"]     # [P, CT] fp8: b_v columns
    bp16_ext = io["bp16"]      # [C] bf16
    out_ext = io["out"]        # [BPC, P, TT, C] f32 packed

    # ---------------- pools ----------------
    const_pool = ctx.enter_context(tc.tile_pool(name="consts", bufs=1))
    xf_pool = ctx.enter_context(tc.tile_pool(name="xf", bufs=2))
    xT_pool = ctx.enter_context(tc.tile_pool(name="xT", bufs=2))
    hT_pool = ctx.enter_context(tc.tile_pool(name="hT", bufs=2))
    qk_pool = ctx.enter_context(tc.tile_pool(name="qk", bufs=2))
    v_pool = ctx.enter_context(tc.tile_pool(name="vv", bufs=2))
    pt_pool = ctx.enter_context(tc.tile_pool(name="pt", bufs=2))
    hTn_pool = ctx.enter_context(tc.tile_pool(name="hTn", bufs=2))
    out_pool = ctx.enter_context(tc.tile_pool(name="outb", bufs=2))
    small = ctx.enter_context(tc.tile_pool(name="small", bufs=4))
    tiny = ctx.enter_context(tc.tile_pool(name="tiny", bufs=1))
    rinv_pool = ctx.enter_context(tc.tile_pool(name="rinv", bufs=2))

    psA = ctx.enter_context(tc.tile_pool(name="psA", bufs=6, space="PSUM"))
    psB = ctx.enter_context(tc.tile_pool(name="psB", bufs=2, space="PSUM"))

    # ---------------- tiny consts via memset (no DMA) ----------------
    ones_1x128 = const_pool.tile([1, P], BF16)
    nc.vector.memset(ones_1x128, 1.0)
    ones8_dr = const_pool.tile([P, 2, 16], F8)
    nc.vector.memset(ones8_dr, 1.0)
    # group mask [128, 2]: partition p -> group p//64, value 1/64 (mean-of-64)
    gmask = const_pool.tile([P, 2], F32)
    nc.vector.memset(gmask, 0.0)
    nc.vector.memset(gmask[0:64, 0:1], 1.0 / 64.0)
    nc.vector.memset(gmask[64:128, 1:2], 1.0 / 64.0)
    # broadcast-back mask [2, 128]: maskT[r, p] = (p//64 == r)
    bmaskT_np = np.zeros((2, P), dtype=np.float32)
    bmaskT_np[0, 0:64] = 1.0
    bmaskT_np[1, 64:128] = 1.0
    bmaskT = const_pool.tile([2, P], F32)
    nc.scalar.dma_start(out=bmaskT, in_=nc.inline_tensor(bmaskT_np, "bmaskT_c").ap())
    eps_sb = const_pool.tile([2, 1], F32)
    nc.vector.memset(eps_sb, EPS)
    shift_sb = const_pool.tile([P, 1], F32)
    nc.vector.memset(shift_sb, -SHIFT)
    magic = const_pool.tile([2, CT], I32)   # fast-rsqrt seed constant
    nc.vector.memset(magic, 0x5F3759DF)

    def xT_load(b):
        """channel-major x[b] on the sync HWDGE ring as 2 half ops (4KB
        contiguous per partition each — each op fans out over all 16 SDMA
        engines at full bandwidth)."""
        xT = xT_pool.tile([P, CT, N], BF16, name=f"xT{b}", tag="xT")
        src_ct = xT_ext[b].rearrange("p (ct n) -> p ct n", ct=CT)
        nc.sync.dma_start(out=xT[:, 0:2, :], in_=src_ct[:, 0:2, :])
        nc.sync.dma_start(out=xT[:, 2:4, :], in_=src_ct[:, 2:4, :])
        return xT

    def xf_load(b):
        """token-major x[b] (residual source), 8KB per partition, one op."""
        xf = xf_pool.tile([P, TT, C], BF16, name=f"xf{b}", tag="xf")
        nc.sync.dma_start(out=xf,
                          in_=xf_ext[b].rearrange("p (t c) -> p t c", t=TT))
        return xf

    # batch 0's xT issues first and owns the full DMA bandwidth; weights and
    # xf queue up behind it on the same FIFO ring
    xts = {0: xT_load(0)}

    # warm the ACT table (exp set — the only set the kernel ever needs, since
    # GN's rsqrt runs on DVE) inside the DMA shadow
    warm = small.tile([2, 1], F32, tag="warm")
    nc.scalar.activation(warm, eps_sb, mybir.ActivationFunctionType.Exp,
                         scale=-0.5)

    # ---------------- weights / consts (fp8, packed layouts) ----------------
    wqkv = const_pool.tile([P, CT, 3 * C], F8)
    nc.sync.dma_start(out=wqkv,
                      in_=wqkv_ext.rearrange("p (kt d) -> p kt d", kt=CT))
    wp = const_pool.tile([P, CT, C], F8)
    nc.sync.dma_start(out=wp, in_=wp_ext.rearrange("p (kt d) -> p kt d", kt=CT))

    c32 = const_pool.tile([P, 16], F32)
    nc.scalar.dma_start(out=c32, in_=c32_ext)
    gamma_sb = c32[:, 0:CT]
    beta_sb = c32[:, CT:2 * CT]
    bqk_cols = c32[:, 8:8 + MQK]

    bv8_cols = const_pool.tile([P, CT], F8)
    nc.scalar.dma_start(out=bv8_cols, in_=c8_ext)

    bp16 = const_pool.tile([1, C], BF16)
    nc.scalar.dma_start(
        out=bp16,
        in_=bass.AP(tensor=bp16_ext.tensor, offset=bp16_ext.offset,
                    ap=[[0, 1], [1, C]]),
    )

    # residual source for batch 0 loads behind the weights on the sync ring
    xfs = {0: xf_load(0)}

    bp_bcast = const_pool.tile([P, C], BF16)

    def emit_bp_fold():
        # bp_bcast = broadcast(bv @ W_p + b_proj); emitted mid-batch-0 so the
        # wp-DMA dependency never stalls the front of the PE queue
        ps_bv = psB.tile([1, 512], F32, tag="ps_small")
        for ct in range(CT):
            nc.tensor.matmul(ps_bv, lhsT=bv8_cols[:, ct:ct + 1],
                             rhs=wp[:, ct, :],
                             start=(ct == 0), stop=(ct == CT - 1))
        bp_eff = const_pool.tile([1, C], BF16)
        nc.vector.tensor_add(bp_eff, ps_bv, bp16)
        ps_bc2 = psA.tile([P, 512], F32, tag="ps")
        nc.tensor.matmul(ps_bc2, lhsT=ones_1x128, rhs=bp_eff,
                         start=True, stop=True)
        nc.scalar.copy(bp_bcast, ps_bc2)

    def stats_part1(b, xT):
        """GroupNorm stats through the rsqrt chain; returns rsm [2,CT,2]
        holding [rstd_g, mean_g] per (group, ct).

        In steady state this is emitted one batch AHEAD (after batch b-1's
        pt section) so its serial chain rides in engine idle windows. The
        rsqrt chain runs on gpsimd (vector for latency-critical batch 0),
        keeping both scalar (no Ln/Exp table thrash) and vector clear."""
        eng = nc.vector if b == 0 else nc.gpsimd
        mv = small.tile([P, CT, 2], F32, tag="mv")  # per-channel [mean, var]
        for ct in range(CT):
            st = small.tile([P, 2, 6], F32, tag="st")
            nc.vector.bn_stats(st[:, 0, :], xT[:, ct, 0:512])
            nc.vector.bn_stats(st[:, 1, :], xT[:, ct, 512:1024])
            nc.vector.bn_aggr(mv[:, ct, :], st)
        q2 = small.tile([P, CT, 2], F32, tag="q2")  # [mean, E[x^2]]
        nc.vector.tensor_mul(q2[:, :, 1], mv[:, :, 0], mv[:, :, 0])
        nc.vector.tensor_add(q2[:, :, 1], q2[:, :, 1], mv[:, :, 1])
        nc.vector.tensor_copy(q2[:, :, 0], mv[:, :, 0])
        ps_st = psB.tile([2, 8], F32, tag="ps_small")  # [g, (ct, stat)]
        nc.tensor.matmul(ps_st, lhsT=gmask, rhs=q2, start=True, stop=True)

        st_sb = small.tile([2, CT, 2], F32, tag="st_sb")
        nc.scalar.copy(st_sb, ps_st)
        gmean = st_sb[:, :, 0]    # [2, 4] group means
        gm2 = st_sb[:, :, 1]      # [2, 4] group E[x^2]
        rsm = small.tile([2, CT, 2], F32, tag="rsm")  # [:,ct,0]=rstd [:,ct,1]=mean
        var24 = rsm[:, :, 0]
        eng.tensor_mul(var24, gmean, gmean)
        eng.tensor_sub(var24, gm2, var24)
        eng.tensor_scalar(out=var24, in0=var24, scalar1=EPS,
                          scalar2=None, op0=mybir.AluOpType.add)
        # rstd = rsqrt(var+eps) via bit-trick seed + 2 Newton steps (8
        # values); the int seed ops stay on vector (Pool rejects shift ops)
        iv = small.tile([2, CT], I32, tag="iv")
        nc.vector.tensor_scalar(out=iv, in0=var24.bitcast(I32), scalar1=1,
                                scalar2=None,
                                op0=mybir.AluOpType.logical_shift_right)
        nc.vector.tensor_sub(iv, magic, iv)
        y = iv.bitcast(F32)
        t1 = small.tile([2, CT], F32, tag="t1")
        for _ in range(2):
            eng.tensor_mul(t1, var24, y)
            eng.tensor_mul(t1, t1, y)
            eng.tensor_scalar(out=t1, in0=t1, scalar1=-0.5, scalar2=1.5,
                              op0=mybir.AluOpType.mult,
                              op1=mybir.AluOpType.add)
            eng.tensor_mul(y, y, t1)
        eng.tensor_copy(var24, y)
        eng.tensor_copy(rsm[:, :, 1], gmean)
        return rsm

    def stats_part2(b, xT, rsm):
        """Broadcast group stats to channels and normalize; returns hT."""
        ps_pc = psB.tile([P, CT, 2], F32, tag="ps_small")  # [rstd_c, mean_c]
        nc.tensor.matmul(ps_pc, lhsT=bmaskT, rhs=rsm, start=True, stop=True)
        A_sb = small.tile([P, CT], F32, tag="A")
        B_sb = small.tile([P, CT], F32, tag="B")
        nc.vector.tensor_mul(A_sb, ps_pc[:, :, 0], gamma_sb)
        nc.vector.tensor_mul(B_sb, ps_pc[:, :, 1], A_sb)
        nc.vector.tensor_sub(B_sb, beta_sb, B_sb)
        # normalize: ct0/1 on vector (feeds the first qkT matmuls asap),
        # ct2/3 on gpsimd (concurrent); batch 0 is latency-critical so
        # everything stays on the faster vector engine
        hT = hT_pool.tile([P, CT, N], F8, name=f"hT{b}", tag="hT")
        for ct in range(CT):
            eng = nc.vector if (ct < 2 or b == 0) else nc.gpsimd
            eng.tensor_scalar(
                out=hT[:, ct, :], in0=xT[:, ct, :],
                scalar1=A_sb[:, ct:ct + 1], scalar2=B_sb[:, ct:ct + 1],
                op0=mybir.AluOpType.mult, op1=mybir.AluOpType.add,
            )
        return hT

    def fold_residual(xf):
        # fold (b_v @ W_p + b_proj) into the residual source
        nc.vector.tensor_add(
            xf, xf,
            bass.AP(tensor=bp_bcast.tensor, offset=bp_bcast.offset,
                    ap=[bp_bcast.ap[0], [0, TT], [1, C]]),
        )

    hTs = {0: stats_part2(0, xts[0], stats_part1(0, xts[0]))}
    rsms = {}

    for b in range(BPC):
        xT = xts.pop(b)
        xf = xfs.pop(b)
        hT = hTs.pop(b)

        # ---------------- qkT = (W_qk)^T @ hT  [d-major, fp8 DR] ----------
        qk = qk_pool.tile([P, MQK, N], F8, name=f"qk{b}", tag="qk")
        for m in range(MQK):
            ps0 = psA.tile([P, 512], F32, tag="ps")
            ps1 = psA.tile([P, 512], F32, tag="ps")
            for kc in (0, 2):
                lw = wqkv[:, kc:kc + 2, m * P:(m + 1) * P]
                nc.tensor.matmul(ps0, lhsT=lw, rhs=hT[:, kc:kc + 2, 0:512],
                                 start=(kc == 0), stop=(kc == 2), perf_mode=DR)
                nc.tensor.matmul(ps1, lhsT=lw, rhs=hT[:, kc:kc + 2, 512:1024],
                                 start=(kc == 0), stop=(kc == 2), perf_mode=DR)
            nc.scalar.activation(qk[:, m, 0:512], ps0,
                                 mybir.ActivationFunctionType.Identity,
                                 bias=bqk_cols[:, m:m + 1])
            nc.scalar.activation(qk[:, m, 512:1024], ps1,
                                 mybir.ActivationFunctionType.Identity,
                                 bias=bqk_cols[:, m:m + 1])

        # prefetch next batch's x behind this batch's compute
        if b + 1 < BPC:
            xts[b + 1] = xT_load(b + 1)
            xfs[b + 1] = xf_load(b + 1)

        # ---------------- v = hT^T @ W_v  [token-major, fp8 DR] -----------
        vv = v_pool.tile([P, TT, C], F8, name=f"vv{b}", tag="vv")
        for m in range(TT):
            ps = psA.tile([P, 512], F32, tag="ps")
            for kc in (0, 2):
                nc.tensor.matmul(ps, lhsT=hT[:, kc:kc + 2, m * P:(m + 1) * P],
                                 rhs=wqkv[:, kc:kc + 2, 1024:1536],
                                 start=(kc == 0), stop=(kc == 2), perf_mode=DR)
            nc.vector.tensor_copy(vv[:, m, :], ps)

        if b == 0:
            emit_bp_fold()
            fold_residual(xf)

        # ---------------- scoresT + exp: pt[keys, queries] (fp8 DR) -------
        pt = pt_pool.tile([P, TT, N], F8, name=f"pt{b}", tag="pt")
        for mk in range(TT):
            ps0 = psA.tile([P, 512], F32, tag="ps")
            ps1 = psA.tile([P, 512], F32, tag="ps")
            for cc in (0, 2):
                lw = qk[:, 4 + cc:4 + cc + 2, mk * P:(mk + 1) * P]  # kT block
                nc.tensor.matmul(ps0, lhsT=lw, rhs=qk[:, cc:cc + 2, 0:512],
                                 start=(cc == 0), stop=(cc == 2), perf_mode=DR)
                nc.tensor.matmul(ps1, lhsT=lw, rhs=qk[:, cc:cc + 2, 512:1024],
                                 start=(cc == 0), stop=(cc == 2), perf_mode=DR)
            nc.scalar.activation(pt[:, mk, 0:512], ps0,
                                 mybir.ActivationFunctionType.Exp,
                                 bias=shift_sb, scale=SCALE)
            nc.scalar.activation(pt[:, mk, 512:1024], ps1,
                                 mybir.ActivationFunctionType.Exp,
                                 bias=shift_sb, scale=SCALE)

        # next batch's GN stats pipelined here: the vector/gpsimd work fills
        # idle windows while PE runs this batch's score matmuls, and the tiny
        # ps_st matmul fills the PE's exp-wait gap before the r-sum chain.
        # (ps_pc + normalize are deferred past hTn so the PE queue never
        # waits on the serial rsqrt chain.)
        if b + 1 < BPC:
            rsms[b + 1] = stats_part1(b + 1, xts[b + 1])
            fold_residual(xfs[b + 1])

        # ---------------- softmax denominator r[q] = sum_keys pt (fp8 DR) -
        ps_r0 = psA.tile([1, 512], F32, tag="ps")
        ps_r1 = psA.tile([1, 512], F32, tag="ps")
        for mk in (0, 2, 4, 6):
            nc.tensor.matmul(ps_r0, lhsT=ones8_dr[:, :, 0:1],
                             rhs=pt[:, mk:mk + 2, 0:512],
                             start=(mk == 0), stop=(mk == 6), perf_mode=DR)
            nc.tensor.matmul(ps_r1, lhsT=ones8_dr[:, :, 0:1],
                             rhs=pt[:, mk:mk + 2, 512:1024],
                             start=(mk == 0), stop=(mk == 6), perf_mode=DR)
        r16 = tiny.tile([1, N], BF16, tag="r16")
        nc.scalar.copy(r16[:, 0:512], ps_r0)
        nc.scalar.copy(r16[:, 512:1024], ps_r1)
        rb = rinv_pool.tile([P, N], F32, name=f"rb{b}", tag="rb")
        for chunk in range(2):
            ps_b = psA.tile([P, 512], F32, tag="ps")
            nc.tensor.matmul(ps_b, lhsT=ones_1x128,
                             rhs=r16[0:1, chunk * 512:(chunk + 1) * 512],
                             start=True, stop=True)
            # 128-partition-parallel fast reciprocal (psum -> sbuf f32)
            nc.vector.reciprocal_approx_fast(
                out=rb[:, chunk * 512:(chunk + 1) * 512], in_=ps_b)

        # ---------------- hTn = (v^T @ pt) * rb  [channel-major, fp8] -----
        hTn = hTn_pool.tile([P, CT, N], F8, name=f"hTn{b}", tag="hTn")
        for mc in range(CT):
            ps0 = psA.tile([P, 512], F32, tag="ps")
            ps1 = psA.tile([P, 512], F32, tag="ps")
            for mk in (0, 2, 4, 6):
                lw = vv[:, mk:mk + 2, mc * P:(mc + 1) * P]
                nc.tensor.matmul(ps0, lhsT=lw, rhs=pt[:, mk:mk + 2, 0:512],
                                 start=(mk == 0), stop=(mk == 6), perf_mode=DR)
                nc.tensor.matmul(ps1, lhsT=lw, rhs=pt[:, mk:mk + 2, 512:1024],
                                 start=(mk == 0), stop=(mk == 6), perf_mode=DR)
            nc.vector.tensor_mul(hTn[:, mc, 0:512], ps0, rb[:, 0:512])
            nc.vector.tensor_mul(hTn[:, mc, 512:1024], ps1, rb[:, 512:1024])

        # second half of next batch's pipelined GN (rsqrt chain has had the
        # whole r/hTn span to finish, so ps_pc is ready to fire)
        if b + 1 < BPC:
            hTs[b + 1] = stats_part2(b + 1, xts[b + 1], rsms.pop(b + 1))

        # ---------------- proj + residual -> out [token-major, fp8 DR] ----
        # outb in bf16: halves DVE add cost and output DMA bytes; the host
        # upcasts to f32 (the reference output magnitude is O(1), so bf16
        # rounding adds ~0.2% relative error)
        outb = out_pool.tile([P, TT, C], BF16, name=f"outb{b}", tag="outb")
        for m in range(TT):
            ps = psA.tile([P, 512], F32, tag="ps")
            for mc in (0, 2):
                nc.tensor.matmul(ps, lhsT=hTn[:, mc:mc + 2, m * P:(m + 1) * P],
                                 rhs=wp[:, mc:mc + 2, :], start=(mc == 0),
                                 stop=(mc == 2), perf_mode=DR)
            nc.vector.tensor_add(outb[:, m, :], ps, xf[:, m, :])
            if m % 2 == 1:
                nc.gpsimd.dma_start(out=out_ext[b][:, m - 1:m + 1, :],
                                    in_=outb[:, m - 1:m + 1, :])


_CACHED_NC = None


def _build_nc():
    global _CACHED_NC
    if _CACHED_NC is not None:
        return _CACHED_NC
    nc = bacc.Bacc("TRN2", target_bir_lowering=False, debug=False,
                   num_devices=NCORES)
    io = {
        "x": nc.dram_tensor("x", [BPC, P, TT * C], BF16,
                            kind="ExternalInput").ap(),
        "xT16": nc.dram_tensor("xT16", [BPC, P, CT * N], BF16,
                               kind="ExternalInput").ap(),
        "wqkv8": nc.dram_tensor("wqkv8", [P, CT * 3 * C], F8,
                                kind="ExternalInput").ap(),
        "wp8": nc.dram_tensor("wp8", [P, CT * C], F8,
                              kind="ExternalInput").ap(),
        "consts32": nc.dram_tensor("consts32", [P, 16], F32,
                                   kind="ExternalInput").ap(),
        "consts8": nc.dram_tensor("consts8", [P, CT], F8,
                                  kind="ExternalInput").ap(),
        "bp16": nc.dram_tensor("bp16", [C], BF16, kind="ExternalInput").ap(),
        "out": nc.dram_tensor("out", [BPC, P, TT, C], BF16,
                              kind="ExternalOutput").ap(),
    }
    with tile.TileContext(nc) as tc:
        with ExitStack() as ctx:
            _build(ctx, tc, io)
    nc.compile()
    _CACHED_NC = nc
    return nc


def _run(inputs: dict, trace: bool = False):
    nc = _build_nc()
    x = np.ascontiguousarray(inputs["x"], dtype=np.float32).reshape(B, N, C)
    x16 = x.astype(ml_dtypes.bfloat16)
    # token-major packed: [B, P, TT*C], partition p <- token t*128+p
    xf_p = np.ascontiguousarray(
        x16.reshape(B, TT, P, C).transpose(0, 2, 1, 3)).reshape(B, P, TT * C)
    # channel-major packed: [B, P, CT*N], partition p <- channel ct*128+p
    xT16_full = np.ascontiguousarray(
        x.transpose(0, 2, 1).reshape(B, CT, P, N).transpose(0, 2, 1, 3)
    ).astype(ml_dtypes.bfloat16).reshape(B, P, CT * N)

    w8 = np.ascontiguousarray(inputs["w_qkv"], dtype=np.float32).astype(
        ml_dtypes.float8_e4m3)
    wqkv_p = np.ascontiguousarray(
        w8.reshape(CT, P, 3 * C).transpose(1, 0, 2)).reshape(P, CT * 3 * C)
    wp8 = np.ascontiguousarray(inputs["w_proj"], dtype=np.float32).astype(
        ml_dtypes.float8_e4m3)
    wp_p = np.ascontiguousarray(
        wp8.reshape(CT, P, C).transpose(1, 0, 2)).reshape(P, CT * C)

    gamma = np.asarray(inputs["gamma"], dtype=np.float32)
    beta = np.asarray(inputs["beta"], dtype=np.float32)
    bqkv = np.asarray(inputs["b_qkv"], dtype=np.float32)
    consts32 = np.zeros((P, 16), dtype=np.float32)
    consts32[:, 0:CT] = gamma.reshape(CT, P).T
    consts32[:, CT:2 * CT] = beta.reshape(CT, P).T
    consts32[:, 8:8 + MQK] = bqkv[:2 * C].reshape(MQK, P).T
    consts8 = np.ascontiguousarray(
        bqkv[2 * C:].astype(ml_dtypes.float8_e4m3).reshape(CT, P).T)

    shared = {
        "wqkv8": wqkv_p,
        "wp8": wp_p,
        "consts32": consts32,
        "consts8": consts8,
        "bp16": np.asarray(inputs["b_proj"], dtype=np.float32)
            .astype(ml_dtypes.bfloat16),
    }
    in_maps = []
    for i in range(NCORES):
        m = {"x": xf_p[i * BPC:(i + 1) * BPC],
             "xT16": xT16_full[i * BPC:(i + 1) * BPC]}
        m.update(shared)
        in_maps.append(m)
    res = run_bass_kernel_spmd(nc, in_maps, list(range(NCORES)), trace=trace)
    outs = [res.results[i]["out"] for i in range(NCORES)]   # [BPC, P, TT, C] bf16
    full = np.concatenate(outs, axis=0)                     # [B, P, TT, C]
    full = full.transpose(0, 2, 1, 3).reshape(B, H, W, C).astype(np.float32)
    return full, res


def kernel(**inputs) -> np.ndarray:
    full, _ = _run(inputs, trace=False)
    return full
